# revision 67
# baseline (speedup 1.0000x reference)
"""Multi-head causal attention (B=2, S=2048, D=1024, H=16) on 8 trn2 cores.

Sharding: batch x head-group (2 batches x 4 groups of 4 heads = 8 cores).

Per-core pipeline (activations bf16, scores matmul fp8 DoubleRow):
  QT/KT = Wq^T x^T, Wk^T x^T   mixed-precision matmuls (contraction chunks
          0-3 in bf16, chunks 4-7 as two fp8 DoubleRow pair-matmuls packing
          256 contraction dims each) -> cast fp8 into a split layout
          qt3/kt3[(h%2)*64+hd, h//2, grp, s] with grp=1 zeroed (DoubleRow
          takes [K, 2, *] operands; the zero band makes group 1 a no-op).
  V     = x @ Wv   bf16, [s, 4 heads, 64+1] with a ones column (denominator).
  S^T   = K Q^T per head: one fp8 DoubleRow matmul per 128-k-tile (2x faster
          than bf16); exp on the Activation engine per k-tile (both heads in
          one instruction); causal mask via tri-multiply on GpSimd.
  O     = P^T-as-weights x V' in natural [q, hd] orientation (65-wide rhs,
          half the PE cost of the transposed form); softmax normalization via
          strided reciprocal + stride-0 broadcast multiply on DVE.
  O^T   via PE transpose of head-pair blocks [128q, 128dl] (bf16).
  Y     = O^T chunks @ Wo (bf16), psum->bf16 copy on DVE, DMA out per
          128-row stripe.  Host sums the 4 group partials per batch in f32
          and adds bo.

A chain of f32 identity matmuls at t~0.7-4us keeps the PE p-state ramp warm
through the initial x/weight DMA window, so real matmuls start at full clock.

Instruction emission is ordered by an event-accurate greedy scheduler: it
tracks per-engine wall clocks plus the score-psum ring and per-k-tile exp
completion times, keeps the Activation engine (the secondary critical path)
fed, drafts deferrable PE work (V chains, out-proj) into the stalls, and
reserves a block of out-proj steps as PE filler for the ACT-bound end phase.
"""

import os
from collections import deque

import numpy as np
import ml_dtypes

import concourse.bacc as bacc
import concourse.mybir as mybir
import concourse.tile as tile
from concourse.bass_utils import run_bass_kernel_spmd
from concourse.masks import make_identity, make_upper_triangular

F32 = mybir.dt.float32
BF16 = mybir.dt.bfloat16
FP8 = mybir.dt.float8e4
EXP = mybir.ActivationFunctionType.Exp
DR = mybir.MatmulPerfMode.DoubleRow
MULT = mybir.AluOpType.mult

B, S, D, H, HD = 2, 2048, 1024, 16, 64
P = 128
KD = D // P          # 8 contraction chunks over D
NJ = S // 512        # 4 q-stages of 512
NHL = 4              # heads per core
DL = NHL * HD        # 256 local head dims
N_CORES = 8

# fp8 contraction chunks for the Q/K projections (out of 8).  Chunks
# [KD-NC8, KD) run as fp8 DoubleRow pair-matmuls; the rest stay bf16.
NC8 = int(os.environ.get("K_FP8_CHUNKS", "4"))
assert NC8 % 2 == 0
NCB = KD - NC8       # bf16 chunks
NP8 = NC8 // 2       # fp8 DR pair-matmuls

PPOOL = int(os.environ.get("K_PPOOL", "52"))
LEAD = float(os.environ.get("K_LEAD", "600"))
SEM = float(os.environ.get("K_SEM", "300"))
SEQ = os.environ.get("K_SEQ", "0") == "1"
LOOKAHEAD = int(os.environ.get("K_LOOKAHEAD", "3"))
N_WARM = int(os.environ.get("K_WARM", "10"))
O_FREE = int(os.environ.get("K_OFREE", "6"))
S_CUT = int(os.environ.get("K_SCUT", "56"))
MSKMM = os.environ.get("K_MSKMM", "1") == "1"   # mask matmul vs Pool tri-mult
SCOPY = os.environ.get("K_SCOPY", "1") == "1"   # late o-copies on Act engine

PE_C = 0.4167
ACT_C = 0.8333

QK_CYC = (NCB * 512 + NP8 * 256)   # PE cycles per qk chain

EMIT_LOG = []


def build_nc(order="G"):
    del order
    EMIT_LOG.clear()
    nc = bacc.Bacc("TRN2", target_bir_lowering=False, debug=False)
    # full bf16 x^T (V projection needs all 8 chunks in bf16); the qk
    # chains additionally get fp8 copies of chunks NCB..8 (xT8) and the
    # matching fp8 weight rows (wqk8) for DoubleRow pair-matmuls.
    xT = nc.dram_tensor("xT", [D, S], BF16, kind="ExternalInput")
    # wqk columns: [q_m0 | k_m0 | q_m1 | k_m1], 128 each
    wqk = nc.dram_tensor("wqk", [NCB * P, 2 * DL], BF16, kind="ExternalInput")
    if NC8:
        xT8 = nc.dram_tensor("xT8", [NC8 * P, S], FP8, kind="ExternalInput")
        wqk8 = nc.dram_tensor("wqk8", [NC8 * P, 2 * DL], FP8,
                              kind="ExternalInput")
    wv = nc.dram_tensor("wv", [D, DL], BF16, kind="ExternalInput")
    wo = nc.dram_tensor("wo", [DL, D], BF16, kind="ExternalInput")
    # msk packs tri2: -192*[64g+p < k] ([64, 2, 128]) and a zero-extended
    # identity id2: [64g+p == w] ([64, 2, 512]).  Per diagonal s-step the
    # chain is: DoubleRow matmul tri2^T @ id2[:, :, 0:W] (start=True, full
    # W-wide region, writes -192 on the masked q<k half of the diag block
    # and 0 elsewhere), then the kq matmul accumulates on top (stop=True).
    # exp(0.125*(s-192)) ~ 1e-10, so no separate mask multiply is needed.
    msk = nc.dram_tensor("msk", [P, 2 * P + 2 * 512], FP8,
                         kind="ExternalInput")
    y = nc.dram_tensor("y", [S, D], BF16, kind="ExternalOutput")

    xT_v = xT.ap().rearrange("(ko p) s -> p ko s", p=P)
    wqk_v = wqk.ap().rearrange("(ko p) (b n) -> p ko b n", p=P, b=4)
    if NC8:
        xT8_v = xT8.ap().rearrange("(c g p) s -> p c g s", p=P, g=2)
        wqk8_v = wqk8.ap().rearrange("(c g p) (b n) -> p c g b n", p=P, g=2,
                                     b=4)
    wv_v = wv.ap().rearrange("(ko p) n -> p ko n", p=P)
    wo_v = wo.ap().rearrange("(ko p) n -> p ko n", p=P)
    mskt_v = msk.ap()[:, 0:2 * P].rearrange("p (g n) -> p g n", g=2)
    mski_v = msk.ap()[:, 2 * P:].rearrange("p (g n) -> p g n", g=2)
    y_v = y.ap()

    with tile.TileContext(nc) as tc:
        with (
            tc.tile_pool(name="singles", bufs=1) as singles,
            tc.tile_pool(name="xpool", bufs=NJ) as xpool,
            tc.tile_pool(name="x8pool", bufs=NJ) as x8pool,
            tc.tile_pool(name="ppool", bufs=PPOOL) as ppool,
            tc.tile_pool(name="opool", bufs=3) as opool,
            tc.tile_pool(name="ypool", bufs=4) as ypool,
            tc.tile_pool(name="recpool", bufs=8) as recpool,
            tc.tile_pool(name="psum", bufs=1, space="PSUM") as psum,
        ):
            # ---- constants ----
            # DMA_ENGINES processes transfers roughly in issue order, so the
            # first-chain critical set (x0-lo on sync, wqk/wqk8 block 0-1 on
            # scalar, xt8 stage 0 on gpsimd) is issued before the bulk
            # weights; the Pool memsets go after the gpsimd DMA triggers so
            # they don't delay the xt8 issue.
            qt3 = singles.tile([P, 2, 2, S], FP8)
            kt3 = singles.tile([P, 2, 2, S], FP8)
            # ident first on the Pool queue: cheap, and the PE warmup
            # chain (which reads ident_f) can start at ~1.2us
            ident_f = singles.tile([P, P], F32)
            make_identity(nc, ident_f[:])
            ident = singles.tile([P, P], BF16)
            nc.vector.tensor_copy(out=ident[:], in_=ident_f[:])
            if not MSKMM:
                tri_f = singles.tile([P, P], F32)
                make_upper_triangular(nc, tri_f[:], val=1.0, diag=True)
                tri = singles.tile([P, P], BF16)
                nc.vector.tensor_copy(out=tri[:], in_=tri_f[:])
            # constants replicated on both partition halves so each head's
            # mask matmul can use the same partition base as its kq matmul
            # (mixed-base accumulation groups fail on hardware)
            mskt_sb = singles.tile([P, 2, P], FP8)
            mski_sb = singles.tile([P, 2, 512], FP8)
            wqk_sb = singles.tile([P, NCB, 4, P], BF16)
            if NC8:
                wqk8_sb = singles.tile([P, NP8, 2, 4, P], FP8)
            wv_sb = singles.tile([P, KD, DL], BF16)
            wo_sb = singles.tile([P, 2, D], BF16)
            # All input DMAs are emitted here in a hand-chosen priority
            # order: DMA_ENGINES serializes transfers roughly in issue
            # order, so the sequence below IS the landing schedule.  x
            # tiles are pre-allocated for all 4 stages (xpool bufs=NJ, no
            # ring reuse).  The zero-band memsets run on Pool between the
            # gpsimd DMA triggers, after the xt8 triggers they must not
            # delay.
            v_sb = singles.tile([P, S // P, NHL, HD + 1], BF16)
            otT_sb = singles.tile([P, 2, S], BF16)
            xt_tiles = {}
            for j in range(NJ):
                xt_tiles[j] = (
                    xpool.tile([P, KD, 512], BF16, name="xt"),
                    x8pool.tile([P, NP8, 2, 512], FP8, name="xt8")
                    if NC8 else None)

            # Per-queue order is FIFO and the DMA engine pool roughly
            # alternates between queues with pending work, so all x
            # transfers go on sync in priority order and the weights on
            # scalar; interleaved landing sequence (approx, +0.9us sem):
            #   x0lo 4.3 | msk 4.4 | wqk8 5.1 | wqk01 5.9 | x08 6.6 |
            #   wqk23 7.3 | x1lo 8.8 | wv 10.2 | x18 11.0 | wo 12.4 |
            #   x0hi 13.9 | x1hi 15.3 | x2lo 16.8 | x28 17.5 | x2hi 19.0 |
            #   x3lo 20.4 | x38 21.2 | x3hi 22.6
            HS = S // 2

            def x_dma(j, part):
                sq0 = 512 * j
                xt, xt8 = xt_tiles[j]
                if part == "lo":
                    nc.sync.dma_start(xt[:, 0:NCB, :],
                                      xT_v[:, 0:NCB, sq0:sq0 + 512])
                elif part == "hi":
                    if NCB < KD:
                        nc.sync.dma_start(xt[:, NCB:KD, :],
                                          xT_v[:, NCB:KD, sq0:sq0 + 512])
                elif NC8:
                    nc.sync.dma_start(xt8[:], xT8_v[:, :, :, sq0:sq0 + 512])

            if MSKMM:
                nc.scalar.dma_start(mskt_sb[:], mskt_v)
                nc.scalar.dma_start(mski_sb[:], mski_v)
            x_dma(0, "lo")
            if NC8:
                nc.sync.dma_start(wqk8_sb[:], wqk8_v[:])
            nc.scalar.dma_start(wqk_sb[:, :, 0:2, :], wqk_v[:, :, 0:2, :])
            x_dma(0, "f8")
            nc.scalar.dma_start(wqk_sb[:, :, 2:4, :], wqk_v[:, :, 2:4, :])
            x_dma(1, "lo")
            nc.scalar.dma_start(wv_sb[:], wv_v)
            x_dma(1, "f8")
            nc.scalar.dma_start(wo_sb[:], wo_v)
            x_dma(0, "hi")
            x_dma(1, "hi")
            x_dma(2, "lo")
            x_dma(2, "f8")
            x_dma(2, "hi")
            x_dma(3, "lo")
            x_dma(3, "f8")
            x_dma(3, "hi")
            nc.gpsimd.memset(kt3[:, :, 1, 0:HS], 0.0)
            nc.gpsimd.memset(qt3[:, :, 1, 0:HS], 0.0)
            nc.gpsimd.memset(kt3[:, :, 1, HS:S], 0.0)
            nc.gpsimd.memset(qt3[:, :, 1, HS:S], 0.0)

            nc.vector.memset(v_sb[:, :, :, HD:HD + 1], 1.0)

            # ---- PE warmup: f32 matmuls (4 cycles/row) spanning the DMA
            # window keep pe_busy_start pinned near 0 so real matmuls run
            # at the full 2.4 GHz clock from the start.
            for _ in range(N_WARM):
                wps = psum.tile([P, P], F32, tag="w", bufs=2, name="warm")
                nc.tensor.matmul(wps[:], ident_f[:], ident_f[:],
                                 start=True, stop=True)

            plists = {(j, pr): [] for j in range(NJ) for pr in range(2)}
            o_tiles = {}

            def emit_qk(j, which, m):
                sq0 = 512 * j
                xt, xt8 = xt_tiles[j]
                bidx = 2 * m + (0 if which == "q" else 1)
                dst = qt3 if which == "q" else kt3
                ps = psum.tile([P, 512], F32, tag="w", bufs=2, name="qk_ps")
                for k in range(NCB):
                    nc.tensor.matmul(
                        ps[:], wqk_sb[:, k, bidx, :], xt[:, k, :],
                        start=(k == 0), stop=(NC8 == 0 and k == NCB - 1))
                for c in range(NP8):
                    nc.tensor.matmul(
                        ps[:], wqk8_sb[:, c, :, bidx, :], xt8[:, c, :, :],
                        start=(NCB == 0 and c == 0), stop=(c == NP8 - 1),
                        perf_mode=DR)
                nc.vector.tensor_copy(out=dst[:, m, 0, sq0:sq0 + 512],
                                      in_=ps[:])

            def emit_v(j, t):
                xt, _ = xt_tiles[j]
                ps = psum.tile([P, DL], F32, tag="w", bufs=2, name="v_ps")
                for k in range(KD):
                    nc.tensor.matmul(
                        ps[:], xt[:, k, 128 * t:128 * t + 128], wv_sb[:, k, :],
                        start=(k == 0), stop=(k == KD - 1))
                nc.vector.tensor_copy(
                    out=v_sb[:, 4 * j + t, :, 0:HD],
                    in_=ps.rearrange("p (h d) -> p h d", h=NHL))

            def emit_s(j, pr, i):
                sq0 = 512 * j
                sq = psum.tile([P, 2, 512], F32, tag="s", bufs=2, name="sq")
                p2 = ppool.tile([P, 2, 512], BF16, name="p2")
                r0 = 128 * i - sq0
                c0 = min(max(r0, 0), 384)
                diag = r0 >= 0
                for hh in range(2):
                    h = 2 * pr + hh
                    base = 64 * (h % 2)
                    hp = h // 2
                    if diag and MSKMM:
                        nc.tensor.matmul(
                            sq[:, hh, c0:512],
                            mskt_sb[base:base + 64, :, :],
                            mski_sb[base:base + 64, :, 0:512 - c0],
                            start=True, stop=False, perf_mode=DR)
                    nc.tensor.matmul(
                        sq[:, hh, c0:512],
                        kt3[base:base + 64, hp, :, 128 * i:128 * i + 128],
                        qt3[base:base + 64, hp, :, sq0 + c0:sq0 + 512],
                        start=not (diag and MSKMM), stop=True, perf_mode=DR)
                nc.scalar.activation(p2[:, :, c0:512], sq[:, :, c0:512],
                                     EXP, scale=0.125)
                if diag and not MSKMM:
                    for hh in range(2):
                        nc.gpsimd.tensor_mul(
                            out=p2[:, hh, r0:r0 + 128],
                            in0=p2[:, hh, r0:r0 + 128], in1=tri[:])
                plists[(j, pr)].append(p2)

            def emit_a(j, pr, t):
                T = 4 * j + t
                plist = plists[(j, pr)]
                if t == 0:
                    o_tiles[(j, pr)] = opool.tile([P, 4, 2, HD], BF16,
                                                  name="o_sb")
                o_sb = o_tiles[(j, pr)]
                u2 = psum.tile([P, 2, HD + 1], F32, tag="u", bufs=2, name="u2")
                for hh in range(2):
                    h = 2 * pr + hh
                    for i in range(T + 1):
                        nc.tensor.matmul(
                            u2[:, hh, :],
                            plist[i][:, hh, 128 * t:128 * t + 128],
                            v_sb[:, i, h, :],
                            start=(i == 0), stop=(i == T))
                rec = recpool.tile([P, 2], F32, name="rec")
                nc.vector.reciprocal(out=rec[:], in_=u2[:, :, HD])
                nc.vector.tensor_tensor(
                    out=o_sb[:, t, :, :], in0=u2[:, :, 0:HD],
                    in1=rec[:, :, None].broadcast_to([P, 2, HD]), op=MULT)

            def emit_tr(j, pr, t):
                sq0 = 512 * j
                o_sb = o_tiles[(j, pr)]
                tp = psum.tile([P, P], BF16, tag="w", bufs=2, name="tp")
                nc.tensor.transpose(tp[:], o_sb[:, t, :, :], ident[:])
                nc.vector.tensor_copy(
                    out=otT_sb[:, pr, sq0 + 128 * t:sq0 + 128 * t + 128],
                    in_=tp[:])

            def emit_o(j, t):
                sq0 = 512 * j
                q0 = sq0 + 128 * t
                late = (j == NJ - 1)
                ysb = ypool.tile([P, D], BF16, name="y_sb")
                for n in range(2):
                    yps = psum.tile([P, 512], F32, tag="w", bufs=2,
                                    name="y_ps")
                    for k in range(2):
                        nc.tensor.matmul(
                            yps[:], otT_sb[:, k, q0:q0 + 128],
                            wo_sb[:, k, 512 * n:512 * n + 512],
                            start=(k == 0), stop=(k == 1))
                    # per-half copy + DMA so the first half ships while the
                    # second computes; final-stage copies alternate onto the
                    # (idle by then) Act engine to unserialize the tail
                    if late and SCOPY and n == 1:
                        nc.scalar.copy(out=ysb[:, 512 * n:512 * n + 512],
                                       in_=yps[:])
                    else:
                        nc.vector.tensor_copy(
                            out=ysb[:, 512 * n:512 * n + 512], in_=yps[:])
                    nc.sync.dma_start(
                        y_v[q0:q0 + 128, 512 * n:512 * n + 512],
                        ysb[:, 512 * n:512 * n + 512])

            def emit(step):
                EMIT_LOG.append((nc.get_next_instruction_name(), step))
                kind = step[0]
                if kind == "x":
                    emit_x(step[1])
                elif kind == "qk":
                    emit_qk(step[1], step[2], step[3])
                elif kind == "v":
                    emit_v(step[1], step[2])
                elif kind == "s":
                    emit_s(step[1], step[2], step[3])
                elif kind == "a":
                    emit_a(step[1], step[2], step[3])
                elif kind == "tr":
                    emit_tr(step[1], step[2], step[3])
                elif kind == "o":
                    emit_o(step[1], step[2])

            if SEQ:
                for j in range(NJ):
                    emit(("x", j))
                    for m in range(2):
                        emit(("qk", j, "q", m))
                        emit(("qk", j, "k", m))
                    for t in range(4):
                        emit(("v", j, t))
                for j in range(NJ):
                    for pr in range(2):
                        for i in range(4 * j + 4):
                            emit(("s", j, pr, i))
                        for t in range(4):
                            emit(("a", j, pr, t))
                            emit(("tr", j, pr, t))
                    for t in range(4):
                        emit(("o", j, t))
                nc.finalize()
                return nc

            # ---- event-accurate greedy scheduler ----
            s_units = [(j, pr) for j in range(NJ) for pr in range(2)]
            s_steps = [(j, pr, i) for (j, pr) in s_units
                       for i in range(4 * j + 4)]

            pe_w = 700.0 + N_WARM * 4 * 128 * 0.8333   # after warmup chain
            act_w = 1500.0
            dve_w = 500.0
            pool_w2 = 13000.0            # memsets (queued after DMA triggers)
            sp_w = 3600.0                # sync queue: x0-lo issued in preamble
            # weight DMA landing estimates (scalar queue: wqk/wqk8 b01,
            # then b23, then x0-hi; gpsimd: xt8(0), wv, wo)
            wqk_land = {("q", 0): 5900.0, ("k", 0): 5900.0,
                        ("q", 1): 7300.0, ("k", 1): 7300.0}
            wv_land = 10200.0
            wo_land = 12400.0
            # all-input DMA landing schedule (preamble priority order)
            xt_land = {0: 6600.0, 1: 11000.0, 2: 17500.0, 3: 21200.0}
            xtv_land = {0: 13900.0, 1: 15300.0, 2: 19000.0, 3: 22600.0}
            sq_ring = deque([0.0, 0.0], maxlen=2)
            exp_end = {u: [] for u in s_units}
            qk_copy = {}
            v_copy = {}
            norm_end = {}
            otT_end = {}
            s_idx = 0
            s_emitted = {u: 0 for u in s_units}
            a_done = {u: 0 for u in s_units}
            qk_done = [[0, 0] for _ in range(NJ)]
            v_done = [0] * NJ
            o_emitted = [0]

            fq = []
            for j in range(NJ):
                for m in range(2):
                    fq.append(("qk", j, "q", m))
                    fq.append(("qk", j, "k", m))
            dq = [("v", j, t) for j in range(NJ) for t in range(4)]
            oq = []
            aq = []
            o_added = set()
            tr_added = set()

            def s_cost_i(j, i):
                r0 = 128 * i - 512 * j
                c0 = min(max(r0, 0), 384)
                pe = (512 - c0) * PE_C * (2.0 if r0 >= 0 and MSKMM else 1.0)
                act = 2 * (512 - c0) * ACT_C + 190.0
                return c0, pe, act

            def v_missing(j, t):
                for jj in range(j):
                    if v_done[jj] < 4:
                        return ("v", jj, v_done[jj])
                if v_done[j] < min(t + 1, 4):
                    return ("v", j, v_done[j])
                return None

            def o_allowed():
                return o_emitted[0] < O_FREE or s_idx >= S_CUT

            def start_of(step):
                kind = step[0]
                cur_j = s_steps[min(s_idx, len(s_steps) - 1)][0]
                if kind == "qk":
                    j, m = step[1], step[3]
                    if j > cur_j + 1:
                        return None
                    return max(pe_w, xt_land[j], wqk_land[(step[2], m)])
                if kind == "v":
                    j = step[1]
                    if j > cur_j + 1:
                        return None
                    return max(pe_w, xtv_land[j], wv_land)
                if kind == "a":
                    j, pr, t = step[1], step[2], step[3]
                    if v_missing(j, t) is not None:
                        return None
                    return max(pe_w, exp_end[(j, pr)][4 * j + t] + SEM)
                if kind == "tr":
                    return max(pe_w, norm_end[(step[1], step[2], step[3])]
                               + SEM)
                if kind == "o":
                    if not o_allowed():
                        return None
                    j, t = step[1], step[2]
                    return max(pe_w, otT_end[(j, 0, t)] + SEM,
                               otT_end[(j, 1, t)] + SEM, wo_land)
                return pe_w

            def run_step(step, st):
                nonlocal pe_w, act_w, dve_w, sp_w
                kind = step[0]
                emit(step)
                if kind == "qk":
                    j, which, m = step[1], step[2], step[3]
                    pe_w = st + QK_CYC * PE_C
                    dve_w = max(dve_w, pe_w + SEM) + 783.0
                    qk_copy[(j, which, m)] = dve_w
                    qk_done[j][m] += 1
                elif kind == "v":
                    j, t = step[1], step[2]
                    pe_w = st + 8 * 256 * PE_C
                    dve_w = max(dve_w, pe_w + SEM) + 517.0
                    v_copy[(j, t)] = dve_w
                    v_done[j] += 1
                elif kind == "s":
                    j, pr, i = step[1], step[2], step[3]
                    c0, pe, act = s_cost_i(j, i)
                    pe_w = st + pe
                    e_st = max(act_w, pe_w + SEM)
                    e_end = e_st + act
                    act_w = e_end
                    sq_ring.append(e_end)
                    if 128 * i - 512 * j >= 0 and not MSKMM:
                        e_end = e_end + SEM + 900.0
                    exp_end[(j, pr)].append(e_end)
                    s_emitted[(j, pr)] += 1
                elif kind == "a":
                    j, pr, t = step[1], step[2], step[3]
                    pe_w = st + 2 * (4 * j + t + 1) * 65 * PE_C
                    dve_w = max(dve_w, pe_w + SEM) + 635.0
                    norm_end[(j, pr, t)] = dve_w
                    a_done[(j, pr)] += 1
                elif kind == "tr":
                    j, pr, t = step[1], step[2], step[3]
                    pe_w = st + 128 * PE_C
                    dve_w = max(dve_w, pe_w + SEM) + 317.0
                    otT_end[(j, pr, t)] = dve_w
                elif kind == "o":
                    j = step[1]
                    pe_w = st + 2048 * PE_C
                    if j == NJ - 1:
                        dve_w = max(dve_w, pe_w + SEM) + 783.0
                        act_w = max(act_w, pe_w + SEM) + 617.0
                        done = max(dve_w, act_w)
                    else:
                        dve_w = max(dve_w, pe_w + SEM) + 2 * 783.0
                        done = dve_w
                    sp_w = max(sp_w, done + SEM) + 2160.0
                    o_emitted[0] += 1

            def s_eligible():
                if s_idx >= len(s_steps):
                    return False
                j, pr, i = s_steps[s_idx]
                for jj in range(j + 1):
                    if (jj, "q", pr) not in qk_copy:
                        return False
                    if (jj, "k", pr) not in qk_copy:
                        return False
                ui = s_units.index((j, pr))
                if ui >= LOOKAHEAD and a_done[s_units[ui - LOOKAHEAD]] < 4:
                    return False
                return True

            def s_start():
                j, pr, i = s_steps[s_idx]
                dep = sq_ring[0] + SEM
                for jj in range(j + 1):
                    dep = max(dep, qk_copy[(jj, "k", pr)] + SEM)
                dep = max(dep, qk_copy[(j, "q", pr)] + SEM)
                return max(pe_w, dep)

            def refresh_queues():
                for u in s_units:
                    j, pr = u
                    n = a_done[u] + sum(1 for q_ in aq
                                        if q_[1] == j and q_[2] == pr)
                    while n < 4 and s_emitted[u] > 4 * j + n:
                        aq.append(("a", j, pr, n))
                        n += 1
                for key in list(norm_end):
                    if key not in tr_added:
                        aq.append(("tr",) + key)
                        tr_added.add(key)
                for j in range(NJ):
                    for t in range(4):
                        if (j, t) in o_added:
                            continue
                        if (j, 0, t) in otT_end and (j, 1, t) in otT_end:
                            oq.append(("o", j, t))
                            o_added.add((j, t))

            def pick_ready(queue, slop):
                for step in queue:
                    st = start_of(step)
                    if st is None:
                        continue
                    if st <= pe_w + slop:
                        return step
                return None

            def remove_step(step):
                for q in (aq, fq, dq, oq):
                    if step in q:
                        q.remove(step)
                        return

            while s_idx < len(s_steps) or fq or dq or aq or oq:
                refresh_queues()
                el = s_eligible()
                sst = s_start() if el else None
                if el and sst <= pe_w + LEAD:
                    step = s_steps[s_idx]
                    s_idx += 1
                    run_step(("s",) + step, sst)
                    continue
                cand = None
                for q in (aq, fq, dq, oq):
                    cand = pick_ready(q, LEAD)
                    if cand is not None:
                        break
                if cand is not None:
                    remove_step(cand)
                    run_step(cand, max(pe_w, start_of(cand)))
                    continue
                # nothing immediately ready: earliest-start option
                options = []
                if el:
                    options.append((sst, None))
                for q in (aq, fq, dq, oq):
                    for step in q:
                        st = start_of(step)
                        if st is not None:
                            options.append((st, step))
                if options:
                    options.sort(key=lambda o: (o[0], o[1] is not None))
                    st, step = options[0]
                    if step is None:
                        sstep = s_steps[s_idx]
                        s_idx += 1
                        run_step(("s",) + sstep, st)
                    else:
                        remove_step(step)
                        run_step(step, st)
                    continue
                # everything blocked on un-emitted prereqs: force x or v,
                # or release the o reserve if it is the only thing left
                forced = None
                for q in (aq, dq):
                    for step in q:
                        if step[0] == "a":
                            miss = v_missing(step[1], step[3])
                            if miss:
                                forced = miss
                                break
                    if forced:
                        break
                if not forced and oq and not o_allowed():
                    o_emitted[0] = -999  # release reserve
                    continue
                if not forced:
                    raise RuntimeError("scheduler stuck")
                remove_step(forced)
                run_step(forced, max(pe_w, start_of(forced) or pe_w))
    nc.finalize()
    return nc


_NC_CACHE = []


def _shard_inputs(inputs):
    bf = ml_dtypes.bfloat16
    f8 = ml_dtypes.float8_e4m3
    x = np.asarray(inputs["x"], dtype=np.float32)
    Wq = np.asarray(inputs["Wq"], dtype=np.float32)
    Wk = np.asarray(inputs["Wk"], dtype=np.float32)
    Wv = np.asarray(inputs["Wv"], dtype=np.float32)
    Wo = np.asarray(inputs["Wo"], dtype=np.float32)
    cut = NCB * P
    in_maps = []
    for c in range(N_CORES):
        b, g = divmod(c, N_CORES // B)
        cols = slice(g * DL, (g + 1) * DL)
        wq_c = Wq[:, cols]
        wk_c = Wk[:, cols]
        wqk = np.concatenate(
            [wq_c[:, 0:128], wk_c[:, 0:128], wq_c[:, 128:256],
             wk_c[:, 128:256]], axis=1)
        xt_full = np.ascontiguousarray(x[b].T)
        dd = (64 * np.arange(2)[None, :, None]
              + np.arange(64)[:, None, None])          # [p, g, 1]
        kk = np.arange(128)[None, None, :]             # [1, 1, k]
        ww = np.arange(512)[None, None, :]             # [1, 1, w]
        tri2 = np.where(dd < kk, np.float32(-192.0), np.float32(0.0))
        id2 = (dd == ww).astype(np.float32)
        msk_half = np.concatenate(
            [tri2.reshape(64, 256), id2.reshape(64, 1024)], axis=1)
        msk_host = np.concatenate([msk_half, msk_half], axis=0)
        m = {
            "xT": xt_full.astype(bf),
            "msk": msk_host.astype(f8),
            "wqk": np.ascontiguousarray(wqk[:cut]).astype(bf),
            "wv": np.ascontiguousarray(Wv[:, cols]).astype(bf),
            "wo": np.ascontiguousarray(Wo[cols, :]).astype(bf),
        }
        if NC8:
            m["xT8"] = np.ascontiguousarray(xt_full[cut:]).astype(f8)
            m["wqk8"] = np.ascontiguousarray(wqk[cut:]).astype(f8)
        in_maps.append(m)
    return in_maps


def kernel(**inputs) -> np.ndarray:
    bo = np.asarray(inputs["bo"], dtype=np.float32)
    in_maps = _shard_inputs(inputs)

    if not _NC_CACHE:
        _NC_CACHE.append(build_nc())
    nc = _NC_CACHE[0]
    res = run_bass_kernel_spmd(nc, in_maps, core_ids=list(range(N_CORES)))
    ys = [np.asarray(r["y"], dtype=np.float32) for r in res.results]
    gpb = N_CORES // B
    out = np.stack([
        np.sum(ys[b * gpb:(b + 1) * gpb], axis=0) + bo for b in range(B)
    ]).astype(np.float32)
    return out


# revision 74
# speedup vs baseline: 1.0047x; 1.0047x over previous
"""Multi-head causal attention (B=2, S=2048, D=1024, H=16) on 8 trn2 cores.

Sharding: batch x head-group (2 batches x 4 groups of 4 heads = 8 cores).

Per-core pipeline (activations bf16, scores matmul fp8 DoubleRow):
  QT/KT = Wq^T x^T, Wk^T x^T   mixed-precision matmuls (contraction chunks
          0-3 in bf16, chunks 4-7 as two fp8 DoubleRow pair-matmuls packing
          256 contraction dims each) -> cast fp8 into a split layout
          qt3/kt3[(h%2)*64+hd, h//2, grp, s] with grp=1 zeroed (DoubleRow
          takes [K, 2, *] operands; the zero band makes group 1 a no-op).
  V     = x @ Wv   bf16, [s, 4 heads, 64+1] with a ones column (denominator).
  S^T   = K Q^T per head: one fp8 DoubleRow matmul per 128-k-tile (2x faster
          than bf16); exp on the Activation engine per k-tile (both heads in
          one instruction); causal mask via tri-multiply on GpSimd.
  O     = P^T-as-weights x V' in natural [q, hd] orientation (65-wide rhs,
          half the PE cost of the transposed form); softmax normalization via
          strided reciprocal + stride-0 broadcast multiply on DVE.
  O^T   via PE transpose of head-pair blocks [128q, 128dl] (bf16).
  Y     = O^T chunks @ Wo (bf16), psum->bf16 copy on DVE, DMA out per
          128-row stripe.  Host sums the 4 group partials per batch in f32
          and adds bo.

A chain of f32 identity matmuls at t~0.7-4us keeps the PE p-state ramp warm
through the initial x/weight DMA window, so real matmuls start at full clock.

Instruction emission is ordered by an event-accurate greedy scheduler: it
tracks per-engine wall clocks plus the score-psum ring and per-k-tile exp
completion times, keeps the Activation engine (the secondary critical path)
fed, drafts deferrable PE work (V chains, out-proj) into the stalls, and
reserves a block of out-proj steps as PE filler for the ACT-bound end phase.
"""

import os
from collections import deque

import numpy as np
import ml_dtypes

import concourse.bacc as bacc
import concourse.mybir as mybir
import concourse.tile as tile
from concourse.bass_utils import run_bass_kernel_spmd
from concourse.masks import make_identity, make_upper_triangular

F32 = mybir.dt.float32
BF16 = mybir.dt.bfloat16
FP8 = mybir.dt.float8e4
EXP = mybir.ActivationFunctionType.Exp
DR = mybir.MatmulPerfMode.DoubleRow
MULT = mybir.AluOpType.mult

B, S, D, H, HD = 2, 2048, 1024, 16, 64
P = 128
KD = D // P          # 8 contraction chunks over D
NJ = S // 512        # 4 q-stages of 512
NHL = 4              # heads per core
DL = NHL * HD        # 256 local head dims
N_CORES = 8

# fp8 contraction chunks for the Q/K projections (out of 8).  Chunks
# [KD-NC8, KD) run as fp8 DoubleRow pair-matmuls; the rest stay bf16.
NC8 = int(os.environ.get("K_FP8_CHUNKS", "4"))
assert NC8 % 2 == 0
NCB = KD - NC8       # bf16 chunks
NP8 = NC8 // 2       # fp8 DR pair-matmuls

PPOOL = int(os.environ.get("K_PPOOL", "52"))
LEAD = float(os.environ.get("K_LEAD", "600"))
SEM = float(os.environ.get("K_SEM", "300"))
SEQ = os.environ.get("K_SEQ", "0") == "1"
LOOKAHEAD = int(os.environ.get("K_LOOKAHEAD", "3"))
N_WARM = int(os.environ.get("K_WARM", "10"))
O_FREE = int(os.environ.get("K_OFREE", "6"))
S_CUT = int(os.environ.get("K_SCUT", "56"))
MSKMM = os.environ.get("K_MSKMM", "1") == "1"   # mask matmul vs Pool tri-mult
SCOPY = os.environ.get("K_SCOPY", "1") == "1"   # late o-copies on Act engine

PE_C = 0.4167
ACT_C = 0.8333

QK_CYC = (NCB * 512 + NP8 * 256)   # PE cycles per qk chain

EMIT_LOG = []


def build_nc(order="G"):
    del order
    EMIT_LOG.clear()
    nc = bacc.Bacc("TRN2", target_bir_lowering=False, debug=False)
    # full bf16 x^T (V projection needs all 8 chunks in bf16); the qk
    # chains additionally get fp8 copies of chunks NCB..8 (xT8) and the
    # matching fp8 weight rows (wqk8) for DoubleRow pair-matmuls.
    xT = nc.dram_tensor("xT", [D, S], BF16, kind="ExternalInput")
    # wqk columns: [q_m0 | k_m0 | q_m1 | k_m1], 128 each
    wqk = nc.dram_tensor("wqk", [NCB * P, 2 * DL], BF16, kind="ExternalInput")
    if NC8:
        xT8 = nc.dram_tensor("xT8", [NC8 * P, S], FP8, kind="ExternalInput")
        wqk8 = nc.dram_tensor("wqk8", [NC8 * P, 2 * DL], FP8,
                              kind="ExternalInput")
    wv = nc.dram_tensor("wv", [D, DL], BF16, kind="ExternalInput")
    wo = nc.dram_tensor("wo", [DL, D], BF16, kind="ExternalInput")
    # msk packs tri2: -192*[64g+p < k] ([64, 2, 128]) and a zero-extended
    # identity id2: [64g+p == w] ([64, 2, 512]).  Per diagonal s-step the
    # chain is: DoubleRow matmul tri2^T @ id2[:, :, 0:W] (start=True, full
    # W-wide region, writes -192 on the masked q<k half of the diag block
    # and 0 elsewhere), then the kq matmul accumulates on top (stop=True).
    # exp(0.125*(s-192)) ~ 1e-10, so no separate mask multiply is needed.
    msk = nc.dram_tensor("msk", [P, 2 * P + 2 * 512], FP8,
                         kind="ExternalInput")
    y = nc.dram_tensor("y", [S, D], BF16, kind="ExternalOutput")

    xT_v = xT.ap().rearrange("(ko p) s -> p ko s", p=P)
    wqk_v = wqk.ap().rearrange("(ko p) (b n) -> p ko b n", p=P, b=4)
    if NC8:
        xT8_v = xT8.ap().rearrange("(c g p) s -> p c g s", p=P, g=2)
        wqk8_v = wqk8.ap().rearrange("(c g p) (b n) -> p c g b n", p=P, g=2,
                                     b=4)
    wv_v = wv.ap().rearrange("(ko p) n -> p ko n", p=P)
    wo_v = wo.ap().rearrange("(ko p) n -> p ko n", p=P)
    mskt_v = msk.ap()[:, 0:2 * P].rearrange("p (g n) -> p g n", g=2)
    mski_v = msk.ap()[:, 2 * P:].rearrange("p (g n) -> p g n", g=2)
    y_v = y.ap()

    with tile.TileContext(nc) as tc:
        with (
            tc.tile_pool(name="singles", bufs=1) as singles,
            tc.tile_pool(name="xpool", bufs=NJ) as xpool,
            tc.tile_pool(name="x8pool", bufs=NJ) as x8pool,
            tc.tile_pool(name="ppool", bufs=PPOOL) as ppool,
            tc.tile_pool(name="opool", bufs=3) as opool,
            tc.tile_pool(name="ypool", bufs=4) as ypool,
            tc.tile_pool(name="recpool", bufs=8) as recpool,
            tc.tile_pool(name="psum", bufs=1, space="PSUM") as psum,
        ):
            # ---- constants ----
            # DMA_ENGINES processes transfers roughly in issue order, so the
            # first-chain critical set (x0-lo on sync, wqk/wqk8 block 0-1 on
            # scalar, xt8 stage 0 on gpsimd) is issued before the bulk
            # weights; the Pool memsets go after the gpsimd DMA triggers so
            # they don't delay the xt8 issue.
            qt3 = singles.tile([P, 2, 2, S], FP8)
            kt3 = singles.tile([P, 2, 2, S], FP8)
            # ident first on the Pool queue: cheap, and the PE warmup
            # chain (which reads ident_f) can start at ~1.2us
            ident_f = singles.tile([P, P], F32)
            make_identity(nc, ident_f[:])
            ident = singles.tile([P, P], BF16)
            nc.vector.tensor_copy(out=ident[:], in_=ident_f[:])
            tri_f = singles.tile([P, P], F32)
            make_upper_triangular(nc, tri_f[:], val=1.0, diag=True)
            tri = singles.tile([P, P], BF16)
            nc.vector.tensor_copy(out=tri[:], in_=tri_f[:])
            # constants replicated on both partition halves so each head's
            # mask matmul can use the same partition base as its kq matmul
            # (mixed-base accumulation groups fail on hardware)
            mskt_sb = singles.tile([P, 2, P], FP8)
            mski_sb = singles.tile([P, 2, 512], FP8)
            wqk_sb = singles.tile([P, NCB, 4, P], BF16)
            if NC8:
                wqk8_sb = singles.tile([P, NP8, 2, 4, P], FP8)
            wv_sb = singles.tile([P, KD, DL], BF16)
            wo_sb = singles.tile([P, 2, D], BF16)
            # All input DMAs are emitted here in a hand-chosen priority
            # order: DMA_ENGINES serializes transfers roughly in issue
            # order, so the sequence below IS the landing schedule.  x
            # tiles are pre-allocated for all 4 stages (xpool bufs=NJ, no
            # ring reuse).  The zero-band memsets run on Pool between the
            # gpsimd DMA triggers, after the xt8 triggers they must not
            # delay.
            v_sb = singles.tile([P, S // P, NHL, HD + 1], BF16)
            otT_sb = singles.tile([P, 2, S], BF16)
            xt_tiles = {}
            for j in range(NJ):
                xt_tiles[j] = (
                    xpool.tile([P, KD, 512], BF16, name="xt"),
                    x8pool.tile([P, NP8, 2, 512], FP8, name="xt8")
                    if NC8 else None)

            # Per-queue order is FIFO and the DMA engine pool roughly
            # alternates between queues with pending work, so all x
            # transfers go on sync in priority order and the weights on
            # scalar; interleaved landing sequence (approx, +0.9us sem):
            #   x0lo 4.3 | msk 4.4 | wqk8 5.1 | wqk01 5.9 | x08 6.6 |
            #   wqk23 7.3 | x1lo 8.8 | wv 10.2 | x18 11.0 | wo 12.4 |
            #   x0hi 13.9 | x1hi 15.3 | x2lo 16.8 | x28 17.5 | x2hi 19.0 |
            #   x3lo 20.4 | x38 21.2 | x3hi 22.6
            HS = S // 2

            def x_dma(j, part):
                sq0 = 512 * j
                xt, xt8 = xt_tiles[j]
                if part == "lo":
                    nc.sync.dma_start(xt[:, 0:NCB, :],
                                      xT_v[:, 0:NCB, sq0:sq0 + 512])
                elif part == "hi":
                    if NCB < KD:
                        nc.sync.dma_start(xt[:, NCB:KD, :],
                                          xT_v[:, NCB:KD, sq0:sq0 + 512])
                elif NC8:
                    nc.sync.dma_start(xt8[:], xT8_v[:, :, :, sq0:sq0 + 512])

            if MSKMM:
                nc.scalar.dma_start(mskt_sb[:], mskt_v)
                nc.scalar.dma_start(mski_sb[:], mski_v)
            x_dma(0, "lo")
            if NC8:
                nc.sync.dma_start(wqk8_sb[:], wqk8_v[:])
            nc.scalar.dma_start(wqk_sb[:, :, 0:2, :], wqk_v[:, :, 0:2, :])
            x_dma(0, "f8")
            nc.scalar.dma_start(wqk_sb[:, :, 2:4, :], wqk_v[:, :, 2:4, :])
            x_dma(1, "lo")
            nc.scalar.dma_start(wv_sb[:], wv_v)
            x_dma(1, "f8")
            nc.scalar.dma_start(wo_sb[:], wo_v)
            x_dma(0, "hi")
            x_dma(1, "hi")
            x_dma(2, "lo")
            x_dma(2, "f8")
            x_dma(2, "hi")
            x_dma(3, "lo")
            x_dma(3, "f8")
            x_dma(3, "hi")
            nc.gpsimd.memset(kt3[:, :, 1, 0:HS], 0.0)
            nc.gpsimd.memset(qt3[:, :, 1, 0:HS], 0.0)
            nc.gpsimd.memset(kt3[:, :, 1, HS:S], 0.0)
            nc.gpsimd.memset(qt3[:, :, 1, HS:S], 0.0)

            nc.vector.memset(v_sb[:, :, :, HD:HD + 1], 1.0)

            # ---- PE warmup: f32 matmuls (4 cycles/row) spanning the DMA
            # window keep pe_busy_start pinned near 0 so real matmuls run
            # at the full 2.4 GHz clock from the start.
            for _ in range(N_WARM):
                wps = psum.tile([P, P], F32, tag="w", bufs=2, name="warm")
                nc.tensor.matmul(wps[:], ident_f[:], ident_f[:],
                                 start=True, stop=True)

            plists = {(j, pr): [] for j in range(NJ) for pr in range(2)}
            o_tiles = {}

            def emit_qk(j, which, m):
                sq0 = 512 * j
                xt, xt8 = xt_tiles[j]
                bidx = 2 * m + (0 if which == "q" else 1)
                dst = qt3 if which == "q" else kt3
                ps = psum.tile([P, 512], F32, tag="w", bufs=2, name="qk_ps")
                for k in range(NCB):
                    nc.tensor.matmul(
                        ps[:], wqk_sb[:, k, bidx, :], xt[:, k, :],
                        start=(k == 0), stop=(NC8 == 0 and k == NCB - 1))
                for c in range(NP8):
                    nc.tensor.matmul(
                        ps[:], wqk8_sb[:, c, :, bidx, :], xt8[:, c, :, :],
                        start=(NCB == 0 and c == 0), stop=(c == NP8 - 1),
                        perf_mode=DR)
                nc.vector.tensor_copy(out=dst[:, m, 0, sq0:sq0 + 512],
                                      in_=ps[:])

            def emit_v(j, t):
                xt, _ = xt_tiles[j]
                ps = psum.tile([P, DL], F32, tag="w", bufs=2, name="v_ps")
                for k in range(KD):
                    nc.tensor.matmul(
                        ps[:], xt[:, k, 128 * t:128 * t + 128], wv_sb[:, k, :],
                        start=(k == 0), stop=(k == KD - 1))
                nc.vector.tensor_copy(
                    out=v_sb[:, 4 * j + t, :, 0:HD],
                    in_=ps.rearrange("p (h d) -> p h d", h=NHL))

            def emit_s(j, pr, i):
                sq0 = 512 * j
                sq = psum.tile([P, 2, 512], F32, tag="s", bufs=2, name="sq")
                p2 = ppool.tile([P, 2, 512], BF16, name="p2")
                r0 = 128 * i - sq0
                c0 = min(max(r0, 0), 384)
                diag = r0 >= 0
                use_mm = MSKMM and j >= NJ - 3
                for hh in range(2):
                    h = 2 * pr + hh
                    base = 64 * (h % 2)
                    hp = h // 2
                    if diag and use_mm:
                        nc.tensor.matmul(
                            sq[:, hh, c0:512],
                            mskt_sb[base:base + 64, :, :],
                            mski_sb[base:base + 64, :, 0:512 - c0],
                            start=True, stop=False, perf_mode=DR)
                    nc.tensor.matmul(
                        sq[:, hh, c0:512],
                        kt3[base:base + 64, hp, :, 128 * i:128 * i + 128],
                        qt3[base:base + 64, hp, :, sq0 + c0:sq0 + 512],
                        start=not (diag and use_mm), stop=True,
                        perf_mode=DR)
                nc.scalar.activation(p2[:, :, c0:512], sq[:, :, c0:512],
                                     EXP, scale=0.125)
                if diag and not use_mm:
                    for hh in range(2):
                        nc.gpsimd.tensor_mul(
                            out=p2[:, hh, r0:r0 + 128],
                            in0=p2[:, hh, r0:r0 + 128], in1=tri[:])
                plists[(j, pr)].append(p2)

            def emit_a(j, pr, t):
                T = 4 * j + t
                plist = plists[(j, pr)]
                if t == 0:
                    o_tiles[(j, pr)] = opool.tile([P, 4, 2, HD], BF16,
                                                  name="o_sb")
                o_sb = o_tiles[(j, pr)]
                u2 = psum.tile([P, 2, HD + 1], F32, tag="u", bufs=2, name="u2")
                for hh in range(2):
                    h = 2 * pr + hh
                    for i in range(T + 1):
                        nc.tensor.matmul(
                            u2[:, hh, :],
                            plist[i][:, hh, 128 * t:128 * t + 128],
                            v_sb[:, i, h, :],
                            start=(i == 0), stop=(i == T))
                rec = recpool.tile([P, 2], F32, name="rec")
                nc.vector.reciprocal(out=rec[:], in_=u2[:, :, HD])
                nc.vector.tensor_tensor(
                    out=o_sb[:, t, :, :], in0=u2[:, :, 0:HD],
                    in1=rec[:, :, None].broadcast_to([P, 2, HD]), op=MULT)

            def emit_tr(j, pr, t):
                sq0 = 512 * j
                o_sb = o_tiles[(j, pr)]
                tp = psum.tile([P, P], BF16, tag="w", bufs=2, name="tp")
                nc.tensor.transpose(tp[:], o_sb[:, t, :, :], ident[:])
                nc.vector.tensor_copy(
                    out=otT_sb[:, pr, sq0 + 128 * t:sq0 + 128 * t + 128],
                    in_=tp[:])

            def emit_o(j, t):
                sq0 = 512 * j
                q0 = sq0 + 128 * t
                late = (j == NJ - 1)
                ysb = ypool.tile([P, D], BF16, name="y_sb")
                for n in range(2):
                    yps = psum.tile([P, 512], F32, tag="w", bufs=2,
                                    name="y_ps")
                    for k in range(2):
                        nc.tensor.matmul(
                            yps[:], otT_sb[:, k, q0:q0 + 128],
                            wo_sb[:, k, 512 * n:512 * n + 512],
                            start=(k == 0), stop=(k == 1))
                    # per-half copy + DMA so the first half ships while the
                    # second computes; final-stage copies alternate onto the
                    # (idle by then) Act engine to unserialize the tail
                    if late and SCOPY and n == 1:
                        nc.scalar.copy(out=ysb[:, 512 * n:512 * n + 512],
                                       in_=yps[:])
                    else:
                        nc.vector.tensor_copy(
                            out=ysb[:, 512 * n:512 * n + 512], in_=yps[:])
                    nc.sync.dma_start(
                        y_v[q0:q0 + 128, 512 * n:512 * n + 512],
                        ysb[:, 512 * n:512 * n + 512])

            def emit(step):
                EMIT_LOG.append((nc.get_next_instruction_name(), step))
                kind = step[0]
                if kind == "x":
                    emit_x(step[1])
                elif kind == "qk":
                    emit_qk(step[1], step[2], step[3])
                elif kind == "v":
                    emit_v(step[1], step[2])
                elif kind == "s":
                    emit_s(step[1], step[2], step[3])
                elif kind == "a":
                    emit_a(step[1], step[2], step[3])
                elif kind == "tr":
                    emit_tr(step[1], step[2], step[3])
                elif kind == "o":
                    emit_o(step[1], step[2])

            if SEQ:
                for j in range(NJ):
                    emit(("x", j))
                    for m in range(2):
                        emit(("qk", j, "q", m))
                        emit(("qk", j, "k", m))
                    for t in range(4):
                        emit(("v", j, t))
                for j in range(NJ):
                    for pr in range(2):
                        for i in range(4 * j + 4):
                            emit(("s", j, pr, i))
                        for t in range(4):
                            emit(("a", j, pr, t))
                            emit(("tr", j, pr, t))
                    for t in range(4):
                        emit(("o", j, t))
                nc.finalize()
                return nc

            # ---- event-accurate greedy scheduler ----
            s_units = [(j, pr) for j in range(NJ) for pr in range(2)]
            s_steps = [(j, pr, i) for (j, pr) in s_units
                       for i in range(4 * j + 4)]

            pe_w = 700.0 + N_WARM * 4 * 128 * 0.8333   # after warmup chain
            act_w = 1500.0
            dve_w = 500.0
            pool_w2 = 13000.0            # memsets (queued after DMA triggers)
            pool_w = [pool_w2]
            sp_w = 3600.0                # sync queue: x0-lo issued in preamble
            # weight DMA landing estimates (scalar queue: wqk/wqk8 b01,
            # then b23, then x0-hi; gpsimd: xt8(0), wv, wo)
            wqk_land = {("q", 0): 5900.0, ("k", 0): 5900.0,
                        ("q", 1): 7300.0, ("k", 1): 7300.0}
            wv_land = 10200.0
            wo_land = 12400.0
            # all-input DMA landing schedule (preamble priority order)
            xt_land = {0: 6600.0, 1: 11000.0, 2: 17500.0, 3: 21200.0}
            xtv_land = {0: 13900.0, 1: 15300.0, 2: 19000.0, 3: 22600.0}
            sq_ring = deque([0.0, 0.0], maxlen=2)
            exp_end = {u: [] for u in s_units}
            qk_copy = {}
            v_copy = {}
            norm_end = {}
            otT_end = {}
            s_idx = 0
            s_emitted = {u: 0 for u in s_units}
            a_done = {u: 0 for u in s_units}
            qk_done = [[0, 0] for _ in range(NJ)]
            v_done = [0] * NJ
            o_emitted = [0]

            fq = []
            for j in range(NJ):
                for m in range(2):
                    fq.append(("qk", j, "q", m))
                    fq.append(("qk", j, "k", m))
            dq = [("v", j, t) for j in range(NJ) for t in range(4)]
            oq = []
            aq = []
            o_added = set()
            tr_added = set()

            def s_cost_i(j, pr, i):
                r0 = 128 * i - 512 * j
                use_mm = MSKMM and j >= NJ - 3
                c0 = min(max(r0, 0), 384)
                pe = (512 - c0) * PE_C * (2.0 if r0 >= 0 and use_mm else 1.0)
                act = 2 * (512 - c0) * ACT_C + 190.0
                return c0, pe, act

            def v_missing(j, t):
                for jj in range(j):
                    if v_done[jj] < 4:
                        return ("v", jj, v_done[jj])
                if v_done[j] < min(t + 1, 4):
                    return ("v", j, v_done[j])
                return None

            def o_allowed():
                return o_emitted[0] < O_FREE or s_idx >= S_CUT

            def start_of(step):
                kind = step[0]
                cur_j = s_steps[min(s_idx, len(s_steps) - 1)][0]
                if kind == "qk":
                    j, m = step[1], step[3]
                    if j > cur_j + 1:
                        return None
                    return max(pe_w, xt_land[j], wqk_land[(step[2], m)])
                if kind == "v":
                    j = step[1]
                    if j > cur_j + 1:
                        return None
                    return max(pe_w, xtv_land[j], wv_land)
                if kind == "a":
                    j, pr, t = step[1], step[2], step[3]
                    if v_missing(j, t) is not None:
                        return None
                    return max(pe_w, exp_end[(j, pr)][4 * j + t] + SEM)
                if kind == "tr":
                    return max(pe_w, norm_end[(step[1], step[2], step[3])]
                               + SEM)
                if kind == "o":
                    if not o_allowed():
                        return None
                    j, t = step[1], step[2]
                    return max(pe_w, otT_end[(j, 0, t)] + SEM,
                               otT_end[(j, 1, t)] + SEM, wo_land)
                return pe_w

            def run_step(step, st):
                nonlocal pe_w, act_w, dve_w, sp_w
                kind = step[0]
                emit(step)
                if kind == "qk":
                    j, which, m = step[1], step[2], step[3]
                    pe_w = st + QK_CYC * PE_C
                    dve_w = max(dve_w, pe_w + SEM) + 783.0
                    qk_copy[(j, which, m)] = dve_w
                    qk_done[j][m] += 1
                elif kind == "v":
                    j, t = step[1], step[2]
                    pe_w = st + 8 * 256 * PE_C
                    dve_w = max(dve_w, pe_w + SEM) + 517.0
                    v_copy[(j, t)] = dve_w
                    v_done[j] += 1
                elif kind == "s":
                    j, pr, i = step[1], step[2], step[3]
                    c0, pe, act = s_cost_i(j, pr, i)
                    pe_w = st + pe
                    e_st = max(act_w, pe_w + SEM)
                    e_end = e_st + act
                    act_w = e_end
                    sq_ring.append(e_end)
                    if (128 * i - 512 * j >= 0
                            and not (MSKMM and j >= NJ - 3)):
                        pool_w[0] = max(pool_w[0], e_end + SEM) + 900.0
                        e_end = pool_w[0]
                    exp_end[(j, pr)].append(e_end)
                    s_emitted[(j, pr)] += 1
                elif kind == "a":
                    j, pr, t = step[1], step[2], step[3]
                    pe_w = st + 2 * (4 * j + t + 1) * 65 * PE_C
                    dve_w = max(dve_w, pe_w + SEM) + 635.0
                    norm_end[(j, pr, t)] = dve_w
                    a_done[(j, pr)] += 1
                elif kind == "tr":
                    j, pr, t = step[1], step[2], step[3]
                    pe_w = st + 128 * PE_C
                    dve_w = max(dve_w, pe_w + SEM) + 317.0
                    otT_end[(j, pr, t)] = dve_w
                elif kind == "o":
                    j = step[1]
                    pe_w = st + 2048 * PE_C
                    if j == NJ - 1:
                        dve_w = max(dve_w, pe_w + SEM) + 783.0
                        act_w = max(act_w, pe_w + SEM) + 617.0
                        done = max(dve_w, act_w)
                    else:
                        dve_w = max(dve_w, pe_w + SEM) + 2 * 783.0
                        done = dve_w
                    sp_w = max(sp_w, done + SEM) + 2160.0
                    o_emitted[0] += 1

            def s_eligible():
                if s_idx >= len(s_steps):
                    return False
                j, pr, i = s_steps[s_idx]
                for jj in range(j + 1):
                    if (jj, "q", pr) not in qk_copy:
                        return False
                    if (jj, "k", pr) not in qk_copy:
                        return False
                ui = s_units.index((j, pr))
                if ui >= LOOKAHEAD and a_done[s_units[ui - LOOKAHEAD]] < 4:
                    return False
                return True

            def s_start():
                j, pr, i = s_steps[s_idx]
                dep = sq_ring[0] + SEM
                for jj in range(j + 1):
                    dep = max(dep, qk_copy[(jj, "k", pr)] + SEM)
                dep = max(dep, qk_copy[(j, "q", pr)] + SEM)
                return max(pe_w, dep)

            def refresh_queues():
                for u in s_units:
                    j, pr = u
                    n = a_done[u] + sum(1 for q_ in aq
                                        if q_[1] == j and q_[2] == pr)
                    while n < 4 and s_emitted[u] > 4 * j + n:
                        aq.append(("a", j, pr, n))
                        n += 1
                for key in list(norm_end):
                    if key not in tr_added:
                        aq.append(("tr",) + key)
                        tr_added.add(key)
                for j in range(NJ):
                    for t in range(4):
                        if (j, t) in o_added:
                            continue
                        if (j, 0, t) in otT_end and (j, 1, t) in otT_end:
                            oq.append(("o", j, t))
                            o_added.add((j, t))

            def pick_ready(queue, slop):
                for step in queue:
                    st = start_of(step)
                    if st is None:
                        continue
                    if st <= pe_w + slop:
                        return step
                return None

            def remove_step(step):
                for q in (aq, fq, dq, oq):
                    if step in q:
                        q.remove(step)
                        return

            while s_idx < len(s_steps) or fq or dq or aq or oq:
                refresh_queues()
                el = s_eligible()
                sst = s_start() if el else None
                if el and sst <= pe_w + LEAD:
                    step = s_steps[s_idx]
                    s_idx += 1
                    run_step(("s",) + step, sst)
                    continue
                cand = None
                for q in (aq, fq, dq, oq):
                    cand = pick_ready(q, LEAD)
                    if cand is not None:
                        break
                if cand is not None:
                    remove_step(cand)
                    run_step(cand, max(pe_w, start_of(cand)))
                    continue
                # nothing immediately ready: earliest-start option
                options = []
                if el:
                    options.append((sst, None))
                for q in (aq, fq, dq, oq):
                    for step in q:
                        st = start_of(step)
                        if st is not None:
                            options.append((st, step))
                if options:
                    options.sort(key=lambda o: (o[0], o[1] is not None))
                    st, step = options[0]
                    if step is None:
                        sstep = s_steps[s_idx]
                        s_idx += 1
                        run_step(("s",) + sstep, st)
                    else:
                        remove_step(step)
                        run_step(step, st)
                    continue
                # everything blocked on un-emitted prereqs: force x or v,
                # or release the o reserve if it is the only thing left
                forced = None
                for q in (aq, dq):
                    for step in q:
                        if step[0] == "a":
                            miss = v_missing(step[1], step[3])
                            if miss:
                                forced = miss
                                break
                    if forced:
                        break
                if not forced and oq and not o_allowed():
                    o_emitted[0] = -999  # release reserve
                    continue
                if not forced:
                    raise RuntimeError("scheduler stuck")
                remove_step(forced)
                run_step(forced, max(pe_w, start_of(forced) or pe_w))
    nc.finalize()
    return nc


_NC_CACHE = []


def _shard_inputs(inputs):
    bf = ml_dtypes.bfloat16
    f8 = ml_dtypes.float8_e4m3
    x = np.asarray(inputs["x"], dtype=np.float32)
    Wq = np.asarray(inputs["Wq"], dtype=np.float32)
    Wk = np.asarray(inputs["Wk"], dtype=np.float32)
    Wv = np.asarray(inputs["Wv"], dtype=np.float32)
    Wo = np.asarray(inputs["Wo"], dtype=np.float32)
    cut = NCB * P
    in_maps = []
    for c in range(N_CORES):
        b, g = divmod(c, N_CORES // B)
        cols = slice(g * DL, (g + 1) * DL)
        wq_c = Wq[:, cols]
        wk_c = Wk[:, cols]
        wqk = np.concatenate(
            [wq_c[:, 0:128], wk_c[:, 0:128], wq_c[:, 128:256],
             wk_c[:, 128:256]], axis=1)
        xt_full = np.ascontiguousarray(x[b].T)
        dd = (64 * np.arange(2)[None, :, None]
              + np.arange(64)[:, None, None])          # [p, g, 1]
        kk = np.arange(128)[None, None, :]             # [1, 1, k]
        ww = np.arange(512)[None, None, :]             # [1, 1, w]
        tri2 = np.where(dd < kk, np.float32(-192.0), np.float32(0.0))
        id2 = (dd == ww).astype(np.float32)
        msk_half = np.concatenate(
            [tri2.reshape(64, 256), id2.reshape(64, 1024)], axis=1)
        msk_host = np.concatenate([msk_half, msk_half], axis=0)
        m = {
            "xT": xt_full.astype(bf),
            "msk": msk_host.astype(f8),
            "wqk": np.ascontiguousarray(wqk[:cut]).astype(bf),
            "wv": np.ascontiguousarray(Wv[:, cols]).astype(bf),
            "wo": np.ascontiguousarray(Wo[cols, :]).astype(bf),
        }
        if NC8:
            m["xT8"] = np.ascontiguousarray(xt_full[cut:]).astype(f8)
            m["wqk8"] = np.ascontiguousarray(wqk[cut:]).astype(f8)
        in_maps.append(m)
    return in_maps


def kernel(**inputs) -> np.ndarray:
    bo = np.asarray(inputs["bo"], dtype=np.float32)
    in_maps = _shard_inputs(inputs)

    if not _NC_CACHE:
        _NC_CACHE.append(build_nc())
    nc = _NC_CACHE[0]
    res = run_bass_kernel_spmd(nc, in_maps, core_ids=list(range(N_CORES)))
    ys = [np.asarray(r["y"], dtype=np.float32) for r in res.results]
    gpb = N_CORES // B
    out = np.stack([
        np.sum(ys[b * gpb:(b + 1) * gpb], axis=0) + bo for b in range(B)
    ]).astype(np.float32)
    return out
